# revision 11
# baseline (speedup 1.0000x reference)
"""Trainium2 Bass kernel for CausalSelfAttention (B=4, T=2048, C=1024, H=16, D=64).

Sharding: tensor-parallel over attention heads — 2 heads per core, 8 cores,
zero collectives. Each core computes QKV for its 2 heads (full token range),
runs causal attention, and produces a partial output projection
(its heads' columns of W_proj); the host sums the 8 partials and adds b_proj.
The mixed value tensor (an output of the module) is emitted per-core and
reassembled on the host.

Per-core dataflow (token-major QKV -> norm/rope -> PE transpose to dim-major
-> scores^T [k,q] -> exp (no max subtraction needed: qk-norm bounds scores)
-> attn@v with a ones-column to accumulate the softmax denominator ->
normalize -> output projection).
"""

import numpy as np
import ml_dtypes

import concourse.bass as bass
from concourse import bacc, mybir, tile, masks
from concourse.bass_utils import run_bass_kernel_spmd

dt = mybir.dt
AF = mybir.ActivationFunctionType
ALU = mybir.AluOpType

B, T, C, H, D = 4, 2048, 1024, 16, 64
NCORES = 8
HPC = H // NCORES          # heads per core
HD = HPC * D               # 128 head dims per core
ROPE_BASE = 10000.0
KC = C // 128              # contraction chunks for qkv proj


def build_module(Bv=B, Tv=T, debug_taps=False):
    """Build + compile the per-core Bass module. Identical on all cores (SPMD);
    only the input data differs per core."""
    NT = Bv * Tv
    TPB = Tv // 128        # token tiles per batch
    JPB = Tv // 512        # 512-wide q groups per batch
    QKW = 3 * HD           # 384 qkv output dims per core

    nc = bacc.Bacc("TRN2", target_bir_lowering=False, debug=False)
    if debug_taps:
        dbg_qT = nc.dram_tensor("dbg_qT", (128, NT), dt.bfloat16, kind="ExternalOutput").ap()
        dbg_kT = nc.dram_tensor("dbg_kT", (128, NT), dt.bfloat16, kind="ExternalOutput").ap()
        dbg_aT = nc.dram_tensor("dbg_aT", (128, NT), dt.bfloat16, kind="ExternalOutput").ap()
        dbg_es = nc.dram_tensor("dbg_es", (128, 1024), dt.bfloat16, kind="ExternalOutput").ap()
        dbg_psO = nc.dram_tensor("dbg_psO", (65, 512), dt.float32, kind="ExternalOutput").ap()

    xT_h = nc.dram_tensor("xT", (C, NT), dt.float32, kind="ExternalInput").ap()
    wq_h = nc.dram_tensor("wqkvT", (C, QKW), dt.float32, kind="ExternalInput").ap()
    br_h = nc.dram_tensor("brow", (1, QKW), dt.float32, kind="ExternalInput").ap()
    on_h = nc.dram_tensor("ones", (1, 128), dt.float32, kind="ExternalInput").ap()
    v1_h = nc.dram_tensor("v1s", (NT, HD), dt.float32, kind="ExternalInput").ap()
    cs_h = nc.dram_tensor("cs", (128, 4 * TPB * 32), dt.float32, kind="ExternalInput").ap()
    mk_h = nc.dram_tensor("masku", (128, 128), dt.bfloat16, kind="ExternalInput").ap()
    wp_h = nc.dram_tensor("wp", (HD, C), dt.bfloat16, kind="ExternalInput").ap()
    outp_h = nc.dram_tensor("outp", (NT, C), dt.float32, kind="ExternalOutput").ap()
    val_h = nc.dram_tensor("val", (NT, HD), dt.float32, kind="ExternalOutput").ap()

    f32r = dt.float32r

    with tile.TileContext(nc) as tc:
        with (
            tc.tile_pool(name="const", bufs=1) as const,
            tc.tile_pool(name="work", bufs=1) as work,
            tc.tile_pool(name="io", bufs=1) as io,
            tc.tile_pool(name="ps", bufs=1, space="PSUM") as ps,
        ):
            # ---- constants / persistent tensors ----
            wq_sb = const.tile([128, KC * QKW], f32r, name="wq_sb")
            for kc in range(KC):
                nc.sync.dma_start(
                    wq_sb[:, kc * QKW:(kc + 1) * QKW],
                    wq_h[kc * 128:(kc + 1) * 128, :].bitcast(f32r),
                )
            br_sb = const.tile([1, QKW], f32r, name="br_sb")
            nc.sync.dma_start(br_sb[:], br_h[:].bitcast(f32r))
            on_sb = const.tile([1, 128], f32r, name="on_sb")
            nc.sync.dma_start(on_sb[:], on_h[:].bitcast(f32r))
            wp_sb = const.tile([HD, C], dt.bfloat16, name="wp_sb")
            nc.sync.dma_start(wp_sb[:], wp_h[:])
            cs_sb = const.tile([128, 4 * TPB * 32], dt.float32, name="cs_sb")
            nc.sync.dma_start(cs_sb[:], cs_h[:])
            mk_sb = const.tile([128, 128], dt.bfloat16, name="mk_sb")
            nc.sync.dma_start(mk_sb[:], mk_h[:])
            id_sb = const.tile([128, 128], dt.bfloat16, name="id_sb")
            masks.make_identity(nc, id_sb[:])
            eps_sb = const.tile([128, 1], dt.float32, name="eps_sb")
            nc.vector.memset(eps_sb[:], 1e-6)

            qT_sb = const.tile([128, NT], dt.bfloat16, name="qT_sb")
            kT_sb = const.tile([128, NT], dt.bfloat16, name="kT_sb")
            aT_sb = const.tile([128, NT], dt.bfloat16, name="aT_sb")

            U = 2 * TPB  # rope groups (q tiles then k tiles)

            for b in range(Bv):
                # ================= A: QKV projection (token-major) ========
                qksb = work.tile([128, 2 * Tv], dt.float32, tag="qksb", bufs=1)
                vaug = work.tile([128, TPB * 130], dt.bfloat16, tag="vaug", bufs=2)
                # ones columns of v_aug (softmax denominator accumulators)
                nc.vector.memset(
                    vaug[:].rearrange("p (t h y) -> p t h y", h=2, y=65)[:, :, :, 64:65],
                    1.0,
                )
                for m2 in range(TPB // 2):  # process token tiles in pairs
                    xt = io.tile([128, KC * 256], f32r, tag="xt", bufs=3)
                    g2 = b * TPB + m2 * 2
                    for kc in range(KC):
                        nc.sync.dma_start(
                            xt[:, kc * 256:(kc + 1) * 256],
                            xT_h[kc * 128:(kc + 1) * 128, g2 * 128:(g2 + 2) * 128].bitcast(f32r),
                        )
                    for mi in range(2):
                        m = m2 * 2 + mi
                        g = b * TPB + m
                        ps_qkv = ps.tile([128, QKW], dt.float32, tag="mm", bufs=2)
                        for kc in range(KC):
                            nc.tensor.matmul(
                                ps_qkv[:],
                                xt[:, kc * 256 + mi * 128: kc * 256 + (mi + 1) * 128],
                                wq_sb[:, kc * QKW:(kc + 1) * QKW],
                                start=(kc == 0),
                                stop=False,
                            )
                        nc.tensor.matmul(ps_qkv[:], on_sb[:], br_sb[:], start=False, stop=True)
                        # evacuate q,k into qksb (q at cols m*128, k at Tv + m*128)
                        nc.scalar.copy(
                            qksb[:].rearrange("p (s t) -> p s t", s=2)[:, :, m * 128:(m + 1) * 128],
                            ps_qkv[:, 0:256].rearrange("p (s u) -> p s u", s=2),
                        )
                        # v: mix with lamb*v1 ((1-lamb) folded into W_v on host)
                        v1t = io.tile([128, HD], dt.float32, tag="v1t", bufs=4)
                        nc.sync.dma_start(v1t[:], v1_h[g * 128:(g + 1) * 128, :])
                        vmix = io.tile([128, HD], dt.float32, tag="vmix", bufs=4)
                        nc.vector.tensor_tensor(vmix[:], ps_qkv[:, 256:384], v1t[:], ALU.add)
                        nc.sync.dma_start(val_h[g * 128:(g + 1) * 128, :], vmix[:])
                        nc.vector.tensor_copy(
                            vaug[:, m * 130:(m + 1) * 130].rearrange("p (h y) -> p h y", y=65)[:, :, 0:64],
                            vmix[:].rearrange("p (h d) -> p h d", d=64),
                        )

                # ================= B: rms-norm + rope =====================
                G = 2 * Tv // 64
                sq = work.tile([128, 2 * Tv], dt.float32, tag="qkn", bufs=1, name="sq")
                nc.vector.tensor_tensor(sq[:], qksb[:], qksb[:], ALU.mult)
                ssum = work.tile([128, G], dt.float32, tag="ss", bufs=1)
                nc.vector.tensor_reduce(
                    ssum[:], sq[:].rearrange("p (g d) -> p g d", d=64),
                    mybir.AxisListType.X, ALU.add,
                )
                sfac = work.tile([128, G], dt.float32, tag="sf", bufs=1)
                nc.scalar.activation(sfac[:], ssum[:], AF.Sqrt, scale=1.0 / 64, bias=eps_sb[:])
                rfac = work.tile([128, G], dt.float32, tag="rf", bufs=1)
                nc.vector.reciprocal(rfac[:], sfac[:])
                qkn = work.tile([128, 2 * Tv], dt.float32, tag="qkn", bufs=1, name="qkn")
                nc.vector.tensor_tensor(
                    qkn[:].rearrange("p (g d) -> p g d", d=64),
                    qksb[:].rearrange("p (g d) -> p g d", d=64),
                    rfac[:].broadcast_to([128, G, 64]),
                    ALU.mult,
                )
                # rope (per head to keep APs 3-dim); cols: u*128 + h*64 + d
                rbf = work.tile([128, 2 * Tv], dt.bfloat16, tag="rbf", bufs=1)
                qk4 = qkn[:].rearrange("p (u h d) -> p u h d", h=2, d=64)
                rb4 = rbf[:].rearrange("p (u h d) -> p u h d", h=2, d=64)
                cosv = cs_sb[:, 0:U * 32].rearrange("p (u f) -> p u f", f=32)
                sinv = cs_sb[:, 2 * TPB * 32:2 * TPB * 32 + U * 32].rearrange("p (u f) -> p u f", f=32)
                for h in range(2):
                    x1 = qk4[:, :, h, 0:32]
                    x2 = qk4[:, :, h, 32:64]
                    t1 = work.tile([128, U * 32], dt.float32, tag="tmp", bufs=2, name="t1")
                    t2 = work.tile([128, U * 32], dt.float32, tag="tmp", bufs=2, name="t2")
                    t1v = t1[:].rearrange("p (u f) -> p u f", f=32)
                    t2v = t2[:].rearrange("p (u f) -> p u f", f=32)
                    nc.vector.tensor_tensor(t1v, x1, cosv, ALU.mult)
                    nc.vector.tensor_tensor(t2v, x2, sinv, ALU.mult)
                    nc.vector.tensor_tensor(rb4[:, :, h, 0:32], t1v, t2v, ALU.add)
                    t3 = work.tile([128, U * 32], dt.float32, tag="tmp", bufs=2, name="t3")
                    t4 = work.tile([128, U * 32], dt.float32, tag="tmp", bufs=2, name="t4")
                    t3v = t3[:].rearrange("p (u f) -> p u f", f=32)
                    t4v = t4[:].rearrange("p (u f) -> p u f", f=32)
                    nc.vector.tensor_tensor(t3v, x2, cosv, ALU.mult)
                    nc.vector.tensor_tensor(t4v, x1, sinv, ALU.mult)
                    nc.vector.tensor_tensor(rb4[:, :, h, 32:64], t3v, t4v, ALU.subtract)

                # ================= C: transpose to dim-major ==============
                for half, dst in ((0, qT_sb), (1, kT_sb)):
                    for p4 in range(TPB // 4):
                        tp = ps.tile([128, 512], dt.bfloat16, tag="mm", bufs=2, name="tp")
                        for t4 in range(4):
                            m = p4 * 4 + t4
                            nc.tensor.transpose(
                                tp[:, t4 * 128:(t4 + 1) * 128],
                                rbf[:, half * Tv + m * 128: half * Tv + (m + 1) * 128],
                                id_sb[:],
                            )
                        nc.vector.tensor_copy(dst[:, b * Tv + p4 * 512: b * Tv + (p4 + 1) * 512], tp[:])

                # ================= D: attention ===========================
                for j in range(JPB):
                    psO = [
                        ps.tile([65, 512], dt.float32, tag="ao", bufs=2, name=f"psO{h}")
                        for h in range(2)
                    ]
                    first_mm = [True, True]  # per head: next attn@v matmul is the bank's first
                    qs = b * Tv + j * 512  # q columns base
                    for ip in range(0, 4 * j + 4, 2):  # k-tile pairs (ip, ip+1)
                        for h in range(2):
                            hs = slice(h * 64, (h + 1) * 64)
                            sc = ps.tile([128, 1024], dt.float32, tag="sc", bufs=2, name="sc")
                            for w in range(2):
                                i = ip + w
                                dd = i - 4 * j
                                kbase = b * Tv + i * 128
                                if dd < 0:
                                    nc.tensor.matmul(
                                        sc[:, w * 512:(w + 1) * 512],
                                        kT_sb[hs, kbase:kbase + 128],
                                        qT_sb[hs, qs:qs + 512],
                                        start=True, stop=True,
                                    )
                                else:
                                    for t in range(dd, 4):
                                        nc.tensor.matmul(
                                            sc[:, w * 512 + t * 128: w * 512 + (t + 1) * 128],
                                            kT_sb[hs, kbase:kbase + 128],
                                            qT_sb[hs, qs + t * 128: qs + (t + 1) * 128],
                                            start=True, stop=True,
                                        )
                            eS = work.tile([128, 1024], dt.bfloat16, tag="es", bufs=3, name="eS")
                            dd0, dd1 = ip - 4 * j, ip + 1 - 4 * j
                            if dd1 <= 0:
                                nc.scalar.activation(eS[:], sc[:], AF.Exp, scale=0.125)
                            else:
                                lo0 = max(dd0, 0) * 128
                                nc.scalar.activation(eS[:, lo0:512], sc[:, lo0:512], AF.Exp, scale=0.125)
                                nc.scalar.activation(
                                    eS[:, 512 + dd1 * 128:1024], sc[:, 512 + dd1 * 128:1024],
                                    AF.Exp, scale=0.125,
                                )
                            # mask diagonal blocks (post-exp: zero out k>q)
                            for w in range(2):
                                dd = ip + w - 4 * j
                                if 0 <= dd <= 3:
                                    dcol = w * 512 + dd * 128
                                    nc.vector.tensor_tensor(
                                        eS[:, dcol:dcol + 128], eS[:, dcol:dcol + 128],
                                        mk_sb[:], ALU.mult,
                                    )
                            if debug_taps and b == 0 and j == 0 and ip == 0 and h == 0:
                                nc.sync.dma_start(dbg_es[:, 0:512], eS[:, 0:512])
                                nc.sync.dma_start(dbg_es[:, 640:1024], eS[:, 640:1024])
                            # attn @ v_aug  (accumulate per q-subtile region)
                            for w in range(2):
                                i = ip + w
                                dd = i - 4 * j
                                vsl = vaug[:, i * 130 + h * 65: i * 130 + (h + 1) * 65]
                                if dd < 0:
                                    nc.tensor.matmul(
                                        psO[h][:], vsl, eS[:, w * 512:(w + 1) * 512],
                                        start=first_mm[h], stop=False,
                                    )
                                    first_mm[h] = False
                                else:
                                    for t in range(dd, 4):
                                        nc.tensor.matmul(
                                            psO[h][:, t * 128:(t + 1) * 128],
                                            vsl, eS[:, w * 512 + t * 128: w * 512 + (t + 1) * 128],
                                            start=first_mm[h], stop=(dd == 3 and t == 3),
                                        )
                                        first_mm[h] = False
                    if debug_taps and b == 0 and j == 0:
                        ps_dbg = io.tile([65, 512], dt.float32, tag="psdbg", bufs=1)
                        nc.vector.tensor_copy(ps_dbg[:], psO[0][:])
                        nc.sync.dma_start(dbg_psO[:], ps_dbg[:])
                    # normalize by softmax denominator (row 64 of psO)
                    for h in range(2):
                        rrow = work.tile([1, 512], dt.float32, tag="rrow", bufs=2)
                        nc.vector.reciprocal(rrow[:], psO[h][64:65, :])
                        rrep = work.tile([64, 512], dt.float32, tag="rrep", bufs=2)
                        nc.gpsimd.partition_broadcast(rrep[:], rrow[:])
                        nc.vector.tensor_tensor(
                            aT_sb[h * 64:(h + 1) * 64, qs:qs + 512],
                            psO[h][0:64, :], rrep[:], ALU.mult,
                        )

                if debug_taps:
                    nc.sync.dma_start(dbg_qT[:, b * Tv:(b + 1) * Tv], qT_sb[:, b * Tv:(b + 1) * Tv])
                    nc.sync.dma_start(dbg_kT[:, b * Tv:(b + 1) * Tv], kT_sb[:, b * Tv:(b + 1) * Tv])
                    nc.sync.dma_start(dbg_aT[:, b * Tv:(b + 1) * Tv], aT_sb[:, b * Tv:(b + 1) * Tv])

                # ================= E: output projection (partial) =========
                for m in range(TPB):
                    g = b * TPB + m
                    psP = ps.tile([128, 1024], dt.float32, tag="sc", bufs=2, name="psP")
                    for nn in range(C // 512):
                        nc.tensor.matmul(
                            psP[:, nn * 512:(nn + 1) * 512],
                            aT_sb[:, g * 128:(g + 1) * 128],
                            wp_sb[:, nn * 512:(nn + 1) * 512],
                            start=True, stop=True,
                        )
                    osb = io.tile([128, C], dt.float32, tag="osb", bufs=3)
                    nc.scalar.copy(osb[:, 0:512], psP[:, 0:512])
                    nc.vector.tensor_copy(osb[:, 512:1024], psP[:, 512:1024])
                    nc.sync.dma_start(outp_h[g * 128:(g + 1) * 128, :], osb[:])

    nc.compile()
    return nc


def host_inputs(x, v1, W_qkv, b_qkv, W_proj, b_proj, lamb, Bv=B, Tv=T):
    """Shard + preprocess full inputs into per-core input maps."""
    NT = Bv * Tv
    TPB = Tv // 128
    lam = float(lamb)

    xT = np.ascontiguousarray(np.asarray(x, np.float32).reshape(NT, C).T)

    # rope tables, token-major per 128-tile: cs[p, u*32+f] with position u*128+p
    pos = (np.arange(TPB)[:, None, None] * 128 + np.arange(128)[None, :, None]).astype(np.float32)
    inv_freq = (1.0 / ROPE_BASE ** (np.arange(0, D, 2, dtype=np.float32) / D))[None, None, :]
    ang = pos * inv_freq                      # [TPB, 128, 32]
    cos_t = np.cos(ang).transpose(1, 0, 2).reshape(128, TPB * 32)
    sin_t = np.sin(ang).transpose(1, 0, 2).reshape(128, TPB * 32)
    cs = np.concatenate([cos_t, cos_t, sin_t, sin_t], axis=1).astype(np.float32)
    cs = np.ascontiguousarray(cs)

    masku = np.triu(np.ones((128, 128), np.float32)).astype(ml_dtypes.bfloat16)
    ones = np.ones((1, 128), np.float32)

    W_qkv = np.asarray(W_qkv, np.float32)
    b_qkv = np.asarray(b_qkv, np.float32)
    W_proj = np.asarray(W_proj, np.float32)
    v1 = np.asarray(v1, np.float32)

    in_maps = []
    for c in range(NCORES):
        r0 = c * HD
        Wq = W_qkv[r0:r0 + HD]
        Wk = W_qkv[C + r0:C + r0 + HD]
        Wv = W_qkv[2 * C + r0:2 * C + r0 + HD] * (1.0 - lam)
        wqkvT = np.ascontiguousarray(np.concatenate([Wq, Wk, Wv], axis=0).T)
        brow = np.concatenate([
            b_qkv[r0:r0 + HD], b_qkv[C + r0:C + r0 + HD],
            b_qkv[2 * C + r0:2 * C + r0 + HD] * (1.0 - lam),
        ])[None, :].astype(np.float32)
        brow = np.ascontiguousarray(brow)
        v1s = np.ascontiguousarray(
            (lam * v1[:, c * HPC:(c + 1) * HPC]).transpose(0, 2, 1, 3).reshape(NT, HD)
        )
        wp = np.ascontiguousarray(W_proj[:, r0:r0 + HD].T).astype(ml_dtypes.bfloat16)
        in_maps.append({
            "xT": xT, "wqkvT": wqkvT, "brow": brow, "ones": ones,
            "v1s": v1s, "cs": cs, "masku": masku, "wp": wp,
        })
    return in_maps


def host_gather(results, b_proj, Bv=B, Tv=T):
    NT = Bv * Tv
    out = np.zeros((NT, C), np.float32)
    for c in range(NCORES):
        out += results[c]["outp"]
    out += np.asarray(b_proj, np.float32)[None, :]
    out = out.reshape(Bv, Tv, C)
    value = np.empty((Bv, H, Tv, D), np.float32)
    for c in range(NCORES):
        value[:, c * HPC:(c + 1) * HPC] = (
            results[c]["val"].reshape(Bv, Tv, HPC, D).transpose(0, 2, 1, 3)
        )
    return out, value


_NC_CACHE = {}


def _get_module(Bv=B, Tv=T):
    key = (Bv, Tv)
    if key not in _NC_CACHE:
        _NC_CACHE[key] = build_module(Bv, Tv)
    return _NC_CACHE[key]


last_results = None


def kernel(x, v1, W_qkv, b_qkv, W_proj, b_proj, lamb, _trace=False):
    global last_results
    nc = _get_module()
    in_maps = host_inputs(x, v1, W_qkv, b_qkv, W_proj, b_proj, lamb)
    if _trace:
        _install_ntff_hook()
    res = run_bass_kernel_spmd(nc, in_maps, core_ids=list(range(NCORES)), trace=_trace)
    last_results = res
    return host_gather(res.results, b_proj)


def _install_ntff_hook():
    """Best-effort NTFF profiling hook for axon (used only when _trace=True)."""
    try:
        import types, sys
        if "antenv.axon_hooks" not in sys.modules:
            mod = types.ModuleType("antenv.axon_hooks")
            _h = [None]
            mod.set_axon_ntff_profile_hook = lambda h: _h.__setitem__(0, h)
            mod.get_axon_ntff_profile_hook = lambda: _h[0]
            sys.modules["antenv.axon_hooks"] = mod
        from antenv.axon_hooks import get_axon_ntff_profile_hook, set_axon_ntff_profile_hook
        if get_axon_ntff_profile_hook() is None:
            from trn_agent_boot.trn_boot import _ntff_profile_via_ctypes
            set_axon_ntff_profile_hook(_ntff_profile_via_ctypes("/opt/axon/libaxon_pjrt.so"))
    except Exception:
        pass


# revision 14
# speedup vs baseline: 1.3808x; 1.3808x over previous
"""Trainium2 Bass kernel for CausalSelfAttention (B=4, T=2048, C=1024, H=16, D=64).

Sharding: tensor-parallel over attention heads — 2 heads per core, 8 cores,
zero collectives. Each core computes QKV for its 2 heads (full token range),
runs causal attention, and produces a partial output projection
(its heads' columns of W_proj); the host sums the 8 partials and adds b_proj.
The mixed value tensor (an output of the module) is emitted per-core and
reassembled on the host.

Per-core dataflow (token-major QKV -> norm/rope -> PE transpose to dim-major
-> scores^T [k,q] -> exp (no max subtraction needed: qk-norm bounds scores)
-> attn@v with a ones-column to accumulate the softmax denominator ->
normalize -> output projection).
"""

import numpy as np
import ml_dtypes

import concourse.bass as bass
from concourse import bacc, mybir, tile, masks
from concourse.bass_utils import run_bass_kernel_spmd

dt = mybir.dt
AF = mybir.ActivationFunctionType
ALU = mybir.AluOpType

B, T, C, H, D = 4, 2048, 1024, 16, 64
NCORES = 8
HPC = H // NCORES          # heads per core
HD = HPC * D               # 128 head dims per core
ROPE_BASE = 10000.0
KC = C // 128              # contraction chunks for qkv proj


def build_module(Bv=B, Tv=T, debug_taps=False):
    """Build + compile the per-core Bass module. Identical on all cores (SPMD);
    only the input data differs per core."""
    NT = Bv * Tv
    TPB = Tv // 128        # token tiles per batch
    JPB = Tv // 512        # 512-wide q groups per batch
    QKW = 3 * HD           # 384 qkv output dims per core

    nc = bacc.Bacc("TRN2", target_bir_lowering=False, debug=False)
    if debug_taps:
        dbg_qT = nc.dram_tensor("dbg_qT", (128, NT), dt.bfloat16, kind="ExternalOutput").ap()
        dbg_kT = nc.dram_tensor("dbg_kT", (128, NT), dt.bfloat16, kind="ExternalOutput").ap()
        dbg_aT = nc.dram_tensor("dbg_aT", (128, NT), dt.bfloat16, kind="ExternalOutput").ap()
        dbg_es = nc.dram_tensor("dbg_es", (128, 1024), dt.bfloat16, kind="ExternalOutput").ap()
        dbg_psO = nc.dram_tensor("dbg_psO", (65, 512), dt.float32, kind="ExternalOutput").ap()

    xT_h = nc.dram_tensor("xT", (C, NT), dt.bfloat16, kind="ExternalInput").ap()
    wq_h = nc.dram_tensor("wqkvT", (C, QKW), dt.bfloat16, kind="ExternalInput").ap()
    br_h = nc.dram_tensor("brow", (1, QKW), dt.bfloat16, kind="ExternalInput").ap()
    on_h = nc.dram_tensor("ones", (1, 128), dt.bfloat16, kind="ExternalInput").ap()
    v1_h = nc.dram_tensor("v1s", (NT, HD), dt.float32, kind="ExternalInput").ap()
    cs_h = nc.dram_tensor("cs", (128, 4 * TPB * 32), dt.float32, kind="ExternalInput").ap()
    mk_h = nc.dram_tensor("masku", (128, 128), dt.bfloat16, kind="ExternalInput").ap()
    wp_h = nc.dram_tensor("wp", (HD, C), dt.bfloat16, kind="ExternalInput").ap()
    outp_h = nc.dram_tensor("outp", (NT, C), dt.float32, kind="ExternalOutput").ap()
    val_h = nc.dram_tensor("val", (NT, HD), dt.float32, kind="ExternalOutput").ap()

    f32r = dt.float32r

    with tile.TileContext(nc) as tc:
        with (
            tc.tile_pool(name="const", bufs=1) as const,
            tc.tile_pool(name="work", bufs=1) as work,
            tc.tile_pool(name="io", bufs=1) as io,
            tc.tile_pool(name="ps", bufs=1, space="PSUM") as ps,
        ):
            # ---- constants / persistent tensors ----
            wq_sb = const.tile([128, KC * QKW], dt.bfloat16, name="wq_sb")
            for kc in range(KC):
                nc.sync.dma_start(
                    wq_sb[:, kc * QKW:(kc + 1) * QKW],
                    wq_h[kc * 128:(kc + 1) * 128, :],
                )
            br_sb = const.tile([1, QKW], dt.bfloat16, name="br_sb")
            nc.sync.dma_start(br_sb[:], br_h[:])
            on_sb = const.tile([1, 128], dt.bfloat16, name="on_sb")
            nc.sync.dma_start(on_sb[:], on_h[:])
            wp_sb = const.tile([HD, C], dt.bfloat16, name="wp_sb")
            nc.sync.dma_start(wp_sb[:], wp_h[:])
            cs_sb = const.tile([128, 4 * TPB * 32], dt.float32, name="cs_sb")
            nc.sync.dma_start(cs_sb[:], cs_h[:])
            mk_sb = const.tile([128, 128], dt.bfloat16, name="mk_sb")
            nc.sync.dma_start(mk_sb[:], mk_h[:])
            id_sb = const.tile([128, 128], dt.bfloat16, name="id_sb")
            masks.make_identity(nc, id_sb[:])
            eps_sb = const.tile([128, 1], dt.float32, name="eps_sb")
            nc.vector.memset(eps_sb[:], 1e-6)

            qT_sb = const.tile([128, NT], dt.bfloat16, name="qT_sb")
            kT_sb = const.tile([128, NT], dt.bfloat16, name="kT_sb")
            aT_sb = const.tile([128, NT], dt.bfloat16, name="aT_sb")

            U = 2 * TPB  # rope groups (q tiles then k tiles)

            for b in range(Bv):
                # ================= A: QKV projection (token-major) ========
                qksb = work.tile([128, 2 * Tv], dt.float32, tag="qksb", bufs=1)
                vaug = work.tile([128, TPB * 130], dt.bfloat16, tag="vaug", bufs=2)
                # ones columns of v_aug (softmax denominator accumulators)
                nc.vector.memset(
                    vaug[:].rearrange("p (t h y) -> p t h y", h=2, y=65)[:, :, :, 64:65],
                    1.0,
                )
                for m4 in range(TPB // 4):  # process token tiles in groups of 4
                    xt = io.tile([128, KC * 512], dt.bfloat16, tag="xt", bufs=2)
                    g4 = b * TPB + m4 * 4
                    for kc in range(KC):
                        nc.sync.dma_start(
                            xt[:, kc * 512:(kc + 1) * 512],
                            xT_h[kc * 128:(kc + 1) * 128, g4 * 128:(g4 + 4) * 128],
                        )
                    v1t = io.tile([128, 512], dt.float32, tag="v1t", bufs=2)
                    nc.sync.dma_start(
                        v1t[:].rearrange("p (m d) -> p m d", d=HD),
                        v1_h[g4 * 128:(g4 + 4) * 128, :].rearrange("(m p) d -> p m d", p=128),
                    )
                    vmix4 = io.tile([128, 512], dt.float32, tag="vmix", bufs=2)
                    for mi in range(4):
                        m = m4 * 4 + mi
                        ps_qkv = ps.tile([128, QKW], dt.float32, tag="mm", bufs=2)
                        for kc in range(KC):
                            nc.tensor.matmul(
                                ps_qkv[:],
                                xt[:, kc * 512 + mi * 128: kc * 512 + (mi + 1) * 128],
                                wq_sb[:, kc * QKW:(kc + 1) * QKW],
                                start=(kc == 0),
                                stop=False,
                            )
                        nc.tensor.matmul(ps_qkv[:], on_sb[:], br_sb[:], start=False, stop=True)
                        # evacuate q,k into qksb (q at cols m*128, k at Tv + m*128)
                        nc.scalar.copy(
                            qksb[:].rearrange("p (s t) -> p s t", s=2)[:, :, m * 128:(m + 1) * 128],
                            ps_qkv[:, 0:256].rearrange("p (s u) -> p s u", s=2),
                        )
                        # v: mix with lamb*v1 ((1-lamb) folded into W_v on host)
                        nc.vector.tensor_tensor(
                            vmix4[:, mi * HD:(mi + 1) * HD], ps_qkv[:, 256:384],
                            v1t[:, mi * HD:(mi + 1) * HD], ALU.add,
                        )
                    nc.sync.dma_start(
                        val_h[g4 * 128:(g4 + 4) * 128, :].rearrange("(m p) d -> p m d", p=128),
                        vmix4[:].rearrange("p (m d) -> p m d", d=HD),
                    )
                    nc.vector.tensor_copy(
                        vaug[:, m4 * 520:(m4 + 1) * 520].rearrange("p (m h y) -> p m h y", m=4, y=65)[:, :, :, 0:64],
                        vmix4[:].rearrange("p (m h d) -> p m h d", h=2, d=64),
                    )

                # ================= B: rms-norm + rope =====================
                G = 2 * Tv // 64
                sq = work.tile([128, 2 * Tv], dt.float32, tag="qkn", bufs=1, name="sq")
                nc.vector.tensor_tensor(sq[:], qksb[:], qksb[:], ALU.mult)
                ssum = work.tile([128, G], dt.float32, tag="ss", bufs=1)
                nc.vector.tensor_reduce(
                    ssum[:], sq[:].rearrange("p (g d) -> p g d", d=64),
                    mybir.AxisListType.X, ALU.add,
                )
                sfac = work.tile([128, G], dt.float32, tag="sf", bufs=1)
                nc.scalar.activation(sfac[:], ssum[:], AF.Sqrt, scale=1.0 / 64, bias=eps_sb[:])
                rfac = work.tile([128, G], dt.float32, tag="rf", bufs=1)
                nc.vector.reciprocal(rfac[:], sfac[:])
                qkn = work.tile([128, 2 * Tv], dt.float32, tag="qkn", bufs=1, name="qkn")
                nc.vector.tensor_tensor(
                    qkn[:].rearrange("p (g d) -> p g d", d=64),
                    qksb[:].rearrange("p (g d) -> p g d", d=64),
                    rfac[:].broadcast_to([128, G, 64]),
                    ALU.mult,
                )
                # rope (per head to keep APs 3-dim); cols: u*128 + h*64 + d
                rbf = work.tile([128, 2 * Tv], dt.bfloat16, tag="rbf", bufs=1)
                qk4 = qkn[:].rearrange("p (u h d) -> p u h d", h=2, d=64)
                rb4 = rbf[:].rearrange("p (u h d) -> p u h d", h=2, d=64)
                cosv = cs_sb[:, 0:U * 32].rearrange("p (u f) -> p u f", f=32)
                sinv = cs_sb[:, 2 * TPB * 32:2 * TPB * 32 + U * 32].rearrange("p (u f) -> p u f", f=32)
                for h in range(2):
                    x1 = qk4[:, :, h, 0:32]
                    x2 = qk4[:, :, h, 32:64]
                    t1 = work.tile([128, U * 32], dt.float32, tag="tmp", bufs=2, name="t1")
                    t2 = work.tile([128, U * 32], dt.float32, tag="tmp", bufs=2, name="t2")
                    t1v = t1[:].rearrange("p (u f) -> p u f", f=32)
                    t2v = t2[:].rearrange("p (u f) -> p u f", f=32)
                    nc.vector.tensor_tensor(t1v, x1, cosv, ALU.mult)
                    nc.vector.tensor_tensor(t2v, x2, sinv, ALU.mult)
                    nc.vector.tensor_tensor(rb4[:, :, h, 0:32], t1v, t2v, ALU.add)
                    t3 = work.tile([128, U * 32], dt.float32, tag="tmp", bufs=2, name="t3")
                    t4 = work.tile([128, U * 32], dt.float32, tag="tmp", bufs=2, name="t4")
                    t3v = t3[:].rearrange("p (u f) -> p u f", f=32)
                    t4v = t4[:].rearrange("p (u f) -> p u f", f=32)
                    nc.vector.tensor_tensor(t3v, x2, cosv, ALU.mult)
                    nc.vector.tensor_tensor(t4v, x1, sinv, ALU.mult)
                    nc.vector.tensor_tensor(rb4[:, :, h, 32:64], t3v, t4v, ALU.subtract)

                # ================= C: transpose to dim-major ==============
                for half, dst in ((0, qT_sb), (1, kT_sb)):
                    for p4 in range(TPB // 4):
                        tp = ps.tile([128, 512], dt.bfloat16, tag="mm", bufs=2, name="tp")
                        for t4 in range(4):
                            m = p4 * 4 + t4
                            nc.tensor.transpose(
                                tp[:, t4 * 128:(t4 + 1) * 128],
                                rbf[:, half * Tv + m * 128: half * Tv + (m + 1) * 128],
                                id_sb[:],
                            )
                        nc.vector.tensor_copy(dst[:, b * Tv + p4 * 512: b * Tv + (p4 + 1) * 512], tp[:])

                # ================= D: attention ===========================
                for j in range(JPB):
                    psO = [
                        ps.tile([65, 512], dt.float32, tag="ao", bufs=2, name=f"psO{h}")
                        for h in range(2)
                    ]
                    first_mm = [True, True]  # per head: next attn@v matmul is the bank's first
                    qs = b * Tv + j * 512  # q columns base
                    for ip in range(0, 4 * j + 4, 2):  # k-tile pairs (ip, ip+1)
                        for h in range(2):
                            hs = slice(h * 64, (h + 1) * 64)
                            sc = ps.tile([128, 1024], dt.float32, tag="sc", bufs=2, name="sc")
                            for w in range(2):
                                i = ip + w
                                dd = i - 4 * j
                                kbase = b * Tv + i * 128
                                if dd < 0:
                                    nc.tensor.matmul(
                                        sc[:, w * 512:(w + 1) * 512],
                                        kT_sb[hs, kbase:kbase + 128],
                                        qT_sb[hs, qs:qs + 512],
                                        start=True, stop=True,
                                    )
                                else:
                                    for t in range(dd, 4):
                                        nc.tensor.matmul(
                                            sc[:, w * 512 + t * 128: w * 512 + (t + 1) * 128],
                                            kT_sb[hs, kbase:kbase + 128],
                                            qT_sb[hs, qs + t * 128: qs + (t + 1) * 128],
                                            start=True, stop=True,
                                        )
                            eS = work.tile([128, 1024], dt.bfloat16, tag="es", bufs=3, name="eS")
                            dd0, dd1 = ip - 4 * j, ip + 1 - 4 * j
                            if dd1 <= 0:
                                nc.scalar.activation(eS[:], sc[:], AF.Exp, scale=0.125)
                            else:
                                lo0 = max(dd0, 0) * 128
                                nc.scalar.activation(eS[:, lo0:512], sc[:, lo0:512], AF.Exp, scale=0.125)
                                nc.scalar.activation(
                                    eS[:, 512 + dd1 * 128:1024], sc[:, 512 + dd1 * 128:1024],
                                    AF.Exp, scale=0.125,
                                )
                            # mask diagonal blocks (post-exp: zero out k>q)
                            for w in range(2):
                                dd = ip + w - 4 * j
                                if 0 <= dd <= 3:
                                    dcol = w * 512 + dd * 128
                                    nc.vector.tensor_tensor(
                                        eS[:, dcol:dcol + 128], eS[:, dcol:dcol + 128],
                                        mk_sb[:], ALU.mult,
                                    )
                            if debug_taps and b == 0 and j == 0 and ip == 0 and h == 0:
                                nc.sync.dma_start(dbg_es[:, 0:512], eS[:, 0:512])
                                nc.sync.dma_start(dbg_es[:, 640:1024], eS[:, 640:1024])
                            # attn @ v_aug  (accumulate per q-subtile region)
                            for w in range(2):
                                i = ip + w
                                dd = i - 4 * j
                                vsl = vaug[:, i * 130 + h * 65: i * 130 + (h + 1) * 65]
                                if dd < 0:
                                    nc.tensor.matmul(
                                        psO[h][:], vsl, eS[:, w * 512:(w + 1) * 512],
                                        start=first_mm[h], stop=False,
                                    )
                                    first_mm[h] = False
                                else:
                                    for t in range(dd, 4):
                                        nc.tensor.matmul(
                                            psO[h][:, t * 128:(t + 1) * 128],
                                            vsl, eS[:, w * 512 + t * 128: w * 512 + (t + 1) * 128],
                                            start=first_mm[h], stop=(dd == 3 and t == 3),
                                        )
                                        first_mm[h] = False
                    if debug_taps and b == 0 and j == 0:
                        ps_dbg = io.tile([65, 512], dt.float32, tag="psdbg", bufs=1)
                        nc.vector.tensor_copy(ps_dbg[:], psO[0][:])
                        nc.sync.dma_start(dbg_psO[:], ps_dbg[:])
                    # normalize by softmax denominator (row 64 of psO)
                    for h in range(2):
                        drow = work.tile([1, 512], dt.float32, tag="drow", bufs=2)
                        nc.vector.tensor_copy(drow[:], psO[h][64:65, :])
                        rrow = work.tile([1, 512], dt.float32, tag="rrow", bufs=2)
                        nc.vector.reciprocal_approx_fast(rrow[:], drow[:])
                        rrep = work.tile([64, 512], dt.float32, tag="rrep", bufs=2)
                        nc.gpsimd.partition_broadcast(rrep[:], rrow[:])
                        nc.vector.tensor_tensor(
                            aT_sb[h * 64:(h + 1) * 64, qs:qs + 512],
                            psO[h][0:64, :], rrep[:], ALU.mult,
                        )

                if debug_taps:
                    nc.sync.dma_start(dbg_qT[:, b * Tv:(b + 1) * Tv], qT_sb[:, b * Tv:(b + 1) * Tv])
                    nc.sync.dma_start(dbg_kT[:, b * Tv:(b + 1) * Tv], kT_sb[:, b * Tv:(b + 1) * Tv])
                    nc.sync.dma_start(dbg_aT[:, b * Tv:(b + 1) * Tv], aT_sb[:, b * Tv:(b + 1) * Tv])

                # ================= E: output projection (partial) =========
                for m in range(TPB):
                    g = b * TPB + m
                    psP = ps.tile([128, 1024], dt.float32, tag="sc", bufs=2, name="psP")
                    for nn in range(C // 512):
                        nc.tensor.matmul(
                            psP[:, nn * 512:(nn + 1) * 512],
                            aT_sb[:, g * 128:(g + 1) * 128],
                            wp_sb[:, nn * 512:(nn + 1) * 512],
                            start=True, stop=True,
                        )
                    osb = io.tile([128, C], dt.float32, tag="osb", bufs=3)
                    nc.scalar.copy(osb[:, 0:512], psP[:, 0:512])
                    nc.vector.tensor_copy(osb[:, 512:1024], psP[:, 512:1024])
                    nc.sync.dma_start(outp_h[g * 128:(g + 1) * 128, :], osb[:])

    nc.compile()
    return nc


def host_inputs(x, v1, W_qkv, b_qkv, W_proj, b_proj, lamb, Bv=B, Tv=T):
    """Shard + preprocess full inputs into per-core input maps."""
    NT = Bv * Tv
    TPB = Tv // 128
    lam = float(lamb)

    xT = np.ascontiguousarray(np.asarray(x, np.float32).reshape(NT, C).T).astype(ml_dtypes.bfloat16)

    # rope tables, token-major per 128-tile: cs[p, u*32+f] with position u*128+p
    pos = (np.arange(TPB)[:, None, None] * 128 + np.arange(128)[None, :, None]).astype(np.float32)
    inv_freq = (1.0 / ROPE_BASE ** (np.arange(0, D, 2, dtype=np.float32) / D))[None, None, :]
    ang = pos * inv_freq                      # [TPB, 128, 32]
    cos_t = np.cos(ang).transpose(1, 0, 2).reshape(128, TPB * 32)
    sin_t = np.sin(ang).transpose(1, 0, 2).reshape(128, TPB * 32)
    cs = np.concatenate([cos_t, cos_t, sin_t, sin_t], axis=1).astype(np.float32)
    cs = np.ascontiguousarray(cs)

    masku = np.triu(np.ones((128, 128), np.float32)).astype(ml_dtypes.bfloat16)
    ones = np.ones((1, 128), ml_dtypes.bfloat16)

    W_qkv = np.asarray(W_qkv, np.float32)
    b_qkv = np.asarray(b_qkv, np.float32)
    W_proj = np.asarray(W_proj, np.float32)
    v1 = np.asarray(v1, np.float32)

    in_maps = []
    for c in range(NCORES):
        r0 = c * HD
        Wq = W_qkv[r0:r0 + HD]
        Wk = W_qkv[C + r0:C + r0 + HD]
        Wv = W_qkv[2 * C + r0:2 * C + r0 + HD] * (1.0 - lam)
        wqkvT = np.ascontiguousarray(np.concatenate([Wq, Wk, Wv], axis=0).T).astype(ml_dtypes.bfloat16)
        brow = np.concatenate([
            b_qkv[r0:r0 + HD], b_qkv[C + r0:C + r0 + HD],
            b_qkv[2 * C + r0:2 * C + r0 + HD] * (1.0 - lam),
        ])[None, :].astype(np.float32)
        brow = np.ascontiguousarray(brow).astype(ml_dtypes.bfloat16)
        v1s = np.ascontiguousarray(
            (lam * v1[:, c * HPC:(c + 1) * HPC]).transpose(0, 2, 1, 3).reshape(NT, HD)
        )
        wp = np.ascontiguousarray(W_proj[:, r0:r0 + HD].T).astype(ml_dtypes.bfloat16)
        in_maps.append({
            "xT": xT, "wqkvT": wqkvT, "brow": brow, "ones": ones,
            "v1s": v1s, "cs": cs, "masku": masku, "wp": wp,
        })
    return in_maps


def host_gather(results, b_proj, Bv=B, Tv=T):
    NT = Bv * Tv
    out = np.zeros((NT, C), np.float32)
    for c in range(NCORES):
        out += results[c]["outp"]
    out += np.asarray(b_proj, np.float32)[None, :]
    out = out.reshape(Bv, Tv, C)
    value = np.empty((Bv, H, Tv, D), np.float32)
    for c in range(NCORES):
        value[:, c * HPC:(c + 1) * HPC] = (
            results[c]["val"].reshape(Bv, Tv, HPC, D).transpose(0, 2, 1, 3)
        )
    return out, value


_NC_CACHE = {}


def _get_module(Bv=B, Tv=T):
    key = (Bv, Tv)
    if key not in _NC_CACHE:
        _NC_CACHE[key] = build_module(Bv, Tv)
    return _NC_CACHE[key]


last_results = None


def kernel(x, v1, W_qkv, b_qkv, W_proj, b_proj, lamb, _trace=False):
    global last_results
    nc = _get_module()
    in_maps = host_inputs(x, v1, W_qkv, b_qkv, W_proj, b_proj, lamb)
    if _trace:
        _install_ntff_hook()
    res = run_bass_kernel_spmd(nc, in_maps, core_ids=list(range(NCORES)), trace=_trace)
    last_results = res
    return host_gather(res.results, b_proj)


def _install_ntff_hook():
    """Best-effort NTFF profiling hook for axon (used only when _trace=True)."""
    try:
        import types, sys
        if "antenv.axon_hooks" not in sys.modules:
            mod = types.ModuleType("antenv.axon_hooks")
            _h = [None]
            mod.set_axon_ntff_profile_hook = lambda h: _h.__setitem__(0, h)
            mod.get_axon_ntff_profile_hook = lambda: _h[0]
            sys.modules["antenv.axon_hooks"] = mod
        from antenv.axon_hooks import get_axon_ntff_profile_hook, set_axon_ntff_profile_hook
        if get_axon_ntff_profile_hook() is None:
            from trn_agent_boot.trn_boot import _ntff_profile_via_ctypes
            set_axon_ntff_profile_hook(_ntff_profile_via_ctypes("/opt/axon/libaxon_pjrt.so"))
    except Exception:
        pass
